# revision 20
# baseline (speedup 1.0000x reference)
"""LoftQ linear (4-bit blockwise dequant + linear + LoRA) on 8 trn2 cores.

out = x @ W^T + bias + 2.0 * (x @ A^T) @ B^T
  W[o,i] = (idx[o,i] * 2/15 - 1) * scales[o, i//64]   (idx = 4-bit nibbles)

Sharding: column-parallel - qweight/scales/bias/lora_B sharded along
out_features (4096 -> 512 per core); x and lora_A replicated; outputs
concatenated on host.

Host prep does the full dequant + rank-16 LoRA fold into a bf16 weight
matrix (weight preprocessing), so the device kernel is a pure GEMM:
  out[512, 2048] = W[4096, 512]^T @ x[4096, 2048] + bias
per core, bf16 inputs, fp32 psum accumulate, bf16 outputs.

Device schedule (per core):
  - 512 matmuls [K=128, M=128, N=512] in two phases of 8 psum banks
    (4 o-tiles x 2 t-chunks), k-outer so DMA supply matches consumption;
    the last KTAIL k's of each phase run group-major so the psum->sbuf
    stores overlap the matmul stream instead of serializing at the end.
  - W on the sync HWDGE queue, x t0 on scalar HWDGE, x t1 on gpsimd
    SWDGE, chunked along k (KCHUNKS) for a fast pipeline head; x t2/t3
    follow on sync/scalar; outputs drain late on both HWDGE queues.
    NOTE: the schedule is sensitive to DMA ring depth - going from 11
    to 12 chunks per queue measured +24us from trigger-capacity stalls.
  - every store has a dedicated staging buffer (opool bufs=8) so no
    store ever waits on an output DMA (out-DMAs queue FIFO behind the
    phase-B x prefetch on the same ring, which is harmless as long as
    nothing but the end-of-kernel barrier waits on them); stores
    alternate DVE / ScalarE so the two at each phase tail run in
    parallel.
  - NWARM dummy matmuls at start open the HAM clock gate while the
    first chunks land.
  - post-Tile stream surgery: consecutive duplicate LDWEIGHTS (the
    t-pair shares one stationary) are removed, which takes the warm
    matmul cadence from 259ns to the 216ns streaming limit. (Keep the
    per-matmul progress-sem increments: measured on HW, removing them
    pushes the cadence back to 259ns.)
"""

import numpy as np
import ml_dtypes

OUT_F = 4096
IN_F = 4096
T = 2048  # 2*1024 tokens
R = 16
NCORES = 8
O_SH = OUT_F // NCORES  # 512
NI = IN_F // 128  # 32 k chunks
NO = O_SH // 128  # 4 o tiles
NT = T // 512  # 4 t chunks
C16 = 2.0 / 15.0
KCHUNKS = [1, 1, 2, 2, 2, 4, 4, 4, 4, 4, 4]  # k-granularity of input DMAs
NWARM = 8  # HAM warm-up matmuls
KTAIL = 8  # trailing k's issued group-major (store overlap)

BF16 = ml_dtypes.bfloat16

_cached = {}


def _dedup_ldweights(nc, mybir):
    """Remove InstLdweights that reload the identical stationary operand
    already in the PE array (no waits/updates attached)."""
    dropped = 0
    for blk in nc.main_func.blocks:
        last_key = None
        to_drop = []
        for inst in blk.instructions:
            if isinstance(inst, mybir.InstLdweights):
                key = inst.ins[0].concise()
                si = inst.sync_info
                clean = si is None or (not si.on_wait and not si.on_update)
                if clean and key == last_key:
                    to_drop.append(inst)
                    continue
                last_key = key
            elif isinstance(inst, mybir.InstMatmult):
                pass  # does not clobber the weight buffer
            elif str(getattr(inst, "engine", "")) == "EngineType.PE":
                if inst.opcode not in ("EventSemaphore", "Drain"):
                    last_key = None
        for inst in to_drop:
            blk.instructions.remove(inst)
        dropped += len(to_drop)
    return dropped


def _build_nc():
    import concourse.bacc as bacc
    import concourse.mybir as mybir
    from concourse.tile import TileContext

    f32 = mybir.dt.float32
    bf16 = mybir.dt.bfloat16
    OP = mybir.AluOpType
    AF = mybir.ActivationFunctionType

    nc = bacc.Bacc("TRN2", target_bir_lowering=False)

    xt = nc.dram_tensor("xt", [128, NT, NI, 512], bf16, kind="ExternalInput")
    wd = nc.dram_tensor("wd", [128, NI, O_SH], bf16, kind="ExternalInput")
    biasd = nc.dram_tensor("biasd", [128, NO], f32, kind="ExternalInput")
    out = nc.dram_tensor("out", [NO, 128, T], bf16, kind="ExternalOutput")

    with TileContext(nc) as tc:
        with (
            tc.tile_pool(name="w", bufs=1) as wpool,
            tc.tile_pool(name="x", bufs=1) as xpool,
            tc.tile_pool(name="cst", bufs=1) as cpool,
            tc.tile_pool(name="outp", bufs=8) as opool,
            tc.tile_pool(name="ps", bufs=8, space="PSUM") as pspool,
        ):
            # --- input DMAs, chunked along k so the pipeline head is short.
            wt = wpool.tile([128, NI, O_SH], bf16, tag="w", name="wt")
            xsb = [
                xpool.tile([128, NI, 512], bf16, tag=f"x{t}", name=f"x{t}")
                for t in range(NT)
            ]
            k0 = 0
            for n in KCHUNKS:
                ks = slice(k0, k0 + n)
                nc.sync.dma_start(out=wt[:, ks, :], in_=wd[:, ks, :])
                nc.scalar.dma_start(out=xsb[0][:, ks, :], in_=xt[:, 0, ks, :])
                k0 += n
            # x t1 on the SWDGE queue: fewer/bigger head chunks (descriptor
            # emission on the Q7 is ~1us per dma, so 11 tiny chunks start slow)
            k0 = 0
            for n in [2, 2, 4, 4, 4, 4, 4, 4, 4]:
                ks = slice(k0, k0 + n)
                nc.gpsimd.dma_start(out=xsb[1][:, ks, :], in_=xt[:, 1, ks, :])
                k0 += n
            btile = cpool.tile([128, NO], f32, tag="bias", name="biassb")
            nc.scalar.dma_start(out=btile[:], in_=biasd[:, :])
            # phase-B x chunks follow on the emptied HWDGE queues; output
            # DMAs queue behind them, which is harmless now that every store
            # has a dedicated staging buffer (nothing waits on an out-DMA
            # until the end-of-kernel barrier)
            for k0 in range(0, NI, 8):
                ks = slice(k0, k0 + 8)
                nc.sync.dma_start(out=xsb[2][:, ks, :], in_=xt[:, 2, ks, :])
                nc.scalar.dma_start(out=xsb[3][:, ks, :], in_=xt[:, 3, ks, :])

            # --- PE warm-up: dummy matmuls on scratch data so the HAM clock
            # gate opens before the first real matmul arrives
            wsc = cpool.tile([128, 512], bf16, tag="wsc", name="wsc")
            nc.vector.memset(wsc[:], 0)
            psc = pspool.tile([128, 512], f32, tag="mm", name="psc")
            for d in range(NWARM):
                nc.tensor.matmul(
                    psc[:], wsc[:, :128], wsc[:],
                    start=(d == 0), stop=(d == NWARM - 1),
                )

            def store(p, tcn, ot, on_act, q):
                o_sb = opool.tile([128, 512], bf16, tag="osb", name=f"osb{tcn}_{ot}")
                if on_act:
                    nc.scalar.add(o_sb[:], p[:], btile[:, ot : ot + 1])
                else:
                    nc.vector.tensor_scalar(
                        o_sb[:], p[:], btile[:, ot : ot + 1], None, OP.add
                    )
                q.dma_start(
                    out=out[ot, :, tcn * 512 : (tcn + 1) * 512], in_=o_sb[:]
                )

            # --- main GEMM: two phases of 8 psum banks (4 o x 2 t).
            # k-outer for the head (supply-paced); o-group-major for the last
            # KTAIL k's, keeping the t-pair adjacent (one LDWEIGHTS feeds
            # both) while each group's stores overlap the next group's MMs.
            for ph in range(2):
                ps = [
                    [
                        pspool.tile([128, 512], f32, tag="mm", name=f"p{ph}_{ot}_{tt}")
                        for tt in range(2)
                    ]
                    for ot in range(NO)
                ]
                for k in range(NI - KTAIL):
                    for ot in range(NO):
                        lhsT = wt[:, k, ot * 128 : (ot + 1) * 128]
                        for tt in range(2):
                            nc.tensor.matmul(
                                ps[ot][tt][:],
                                lhsT,
                                xsb[2 * ph + tt][:, k, :],
                                start=(k == 0),
                                stop=False,
                            )
                for ot in range(NO):
                    for k in range(NI - KTAIL, NI):
                        lhsT = wt[:, k, ot * 128 : (ot + 1) * 128]
                        for tt in range(2):
                            nc.tensor.matmul(
                                ps[ot][tt][:],
                                lhsT,
                                xsb[2 * ph + tt][:, k, :],
                                start=False,
                                stop=(k == NI - 1),
                            )
                    for tt in range(2):
                        # phase A outs must use the idle scalar ring; phase B
                        # may fan out to both HWDGE rings
                        q = nc.scalar if (ph == 0 or tt == 0) else nc.sync
                        store(ps[ot][tt], 2 * ph + tt, ot, tt == 1, q)
    _dedup_ldweights(nc, mybir)
    nc.compile()
    return nc


def _pack_rows(a, nblk):
    """[nblk*128, F] -> [128, nblk, F] with blk j, partition p = row j*128+p."""
    f = a.shape[1]
    return np.ascontiguousarray(a.reshape(nblk, 128, f).transpose(1, 0, 2))


def _dequant_full(qweight, scales, lora_A, lora_B):
    """Full [OUT_F, IN_F] f32 weight: 4-bit dequant + LoRA fold."""
    q = qweight.astype(np.int32)
    lo = q & 15
    hi = (q >> 4) & 15
    idx = np.stack([lo, hi], axis=-1).reshape(-1)
    vals = idx.astype(np.float32) * C16 - 1.0
    w = vals.reshape(-1, 64) * scales.reshape(-1, 1).astype(np.float32)
    w = w.reshape(OUT_F, IN_F)
    w += 2.0 * (lora_B.astype(np.float32) @ lora_A.astype(np.float32))
    return w


def prep_inputs(x, qweight, scales, bias, lora_A, lora_B):
    """Host-side dequant + layout prep + sharding. Per-core input maps."""
    x2d = np.ascontiguousarray(x.reshape(T, IN_F)).T  # [IN_F, T]
    xb = _pack_rows(x2d, NI)  # [128, NI, T]
    xb = np.ascontiguousarray(
        xb.reshape(128, NI, NT, 512).transpose(0, 2, 1, 3)
    ).astype(BF16)  # [128, NT, NI, 512]

    w = _dequant_full(qweight, scales, lora_A, lora_B)  # [OUT_F, IN_F]

    in_maps = []
    for c in range(NCORES):
        o0, o1 = c * O_SH, (c + 1) * O_SH
        wc = _pack_rows(np.ascontiguousarray(w[o0:o1].T), NI).astype(BF16)
        bias_c = np.ascontiguousarray(
            bias[o0:o1].reshape(NO, 128).T
        ).astype(np.float32)  # [128, NO]
        in_maps.append({"xt": xb, "wd": wc, "biasd": bias_c})
    return in_maps


def run(in_maps, trace=False):
    from concourse import bass_utils

    if "nc" not in _cached:
        _cached["nc"] = _build_nc()
    res = bass_utils.run_bass_kernel_spmd(
        _cached["nc"], in_maps, list(range(NCORES)), trace=trace
    )
    return res


def assemble(results):
    full = np.concatenate(
        [
            np.asarray(r["out"]).reshape(O_SH, T).astype(np.float32)
            for r in results
        ],
        axis=0,
    )  # [OUT_F, T]
    return np.ascontiguousarray(full.T).reshape(2, 1024, OUT_F)


def kernel(x, qweight, scales, bias, lora_A, lora_B):
    in_maps = prep_inputs(x, qweight, scales, bias, lora_A, lora_B)
    res = run(in_maps, trace=False)
    return assemble(res.results)
